# revision 33
# baseline (speedup 1.0000x reference)
"""Trainium2 Bass kernel for ChebyNet (K=1) forward pass.

ChebConv with K=1 reduces to a plain linear layer on the T0 (identity) term,
so edge_index / edge_weight never enter the math. The network is:

    h1 = x @ W1.T (+b1; cancels in BN) -> BN (train, over nodes) -> ReLU
    h2 = h1 @ W2.T (+b2; cancels)      -> BN -> ReLU
    h3 = relu(h2 @ Wl1.T + bl1)
    out = log_softmax(h3 @ Wl2.T + bl2, axis=1)

Sharding: nodes (N=50000) split across 8 NeuronCores (6250 rows each).
All compute is node-local except BN statistics:
  - BN1 stats come analytically from the Gram matrix of [x | 1] (one
    AllReduce of [128,129] f32 + local math).
  - BN2 stats from per-shard sum/sumsq of h2: AllGather [128,16] + reduce.

v6 changes vs v2 (434925 ns baseline):
  - BN1 stats projected locally before the collective (wxm/e2 are
    linear in the per-core Gram), shrinking collective #1 from the raw
    [128,129] Gram (66KB AllReduce RDH, 14.9us) to [128,16] (8KB
    AllGather Mesh, ~6.4us) + local tree-reduce.
  - ph2 bufs 2->4: cross-m PSUM double-buffering puts the L2 phase at
    its 263ns/matmul cadence floor (p90 delta 264ns).
  - main-phase engine rebalance: h2 copy+sum split ACT/DVE, squares
    split Pool(gpsimd)/DVE; the tail chunk that feeds the AR2 trigger
    splits ACT/DVE so no single queue serializes the collective start.
  - BN2 stat partials laid out in one tile -> single final reduce op;
    AllGather + tree-reduce instead of Mesh AllReduce (~4us); the
    gathered-output copy rides the scalar engine's HW DMA queue (the
    gpsimd software queue adds ~2.4us completion latency).
  - ACT sqrt table pre-warmed at startup (saves a 1.3us table load on
    the BN1 critics path); first group's BN2-apply runs in 512-wide
    halves so L3 starts sooner after the collective.
  - output stores ride the sync+scalar HW DMA queues (the gpsimd
    software queue added a multi-us DRAIN to the epilogue).

Known dead ends (measured): fp8 fails the 2e-2 gate (2.7e-2 L2-only);
a warm-up dummy collective serializes in front of AR1 instead of
hiding the rendezvous barrier (+16us); GPSIMD elementwise BN-apply is
2.6 cyc/elem + steals the DVE SBUF port (final pass 3x slower);
deferring ALL stores past the last group exposes the ~4GB/s/queue
store drain (+24us); matmul outputs cannot span PSUM banks (512 f32
cols max); tensor_tensor_reduce fails walrus codegen.
"""

import os
import sys

sys.path.insert(0, "/opt/trn_rl_repo")

import numpy as np

NCORES = 8
N_TOTAL = 50000
R = N_TOTAL // NCORES  # 6250 rows per core
DIN = 128
H = 1024
HM = 256
C = 10
BN_EPS = 1e-5
CH = 512
NT = (R + 127) // 128  # 49 row tiles (for Gram); host pads x to NT*128 rows
RPAD = NT * 128  # 6272

CH_LIST = [(i * CH, min(CH, R - i * CH)) for i in range((R + CH - 1) // CH)]
NCH = len(CH_LIST)  # 13
# groups of two chunks (last group is the lone tail chunk)
GROUPS = [CH_LIST[i : i + 2] for i in range(0, NCH, 2)]

_CACHE = {}


def _build(stage="full"):
    import concourse.bass as bass  # noqa: F401
    import concourse.tile as tile
    import concourse.mybir as mybir
    from concourse import bacc
    from concourse.masks import make_identity

    fp32 = mybir.dt.float32
    bf16 = mybir.dt.bfloat16
    AF = mybir.ActivationFunctionType
    ALU = mybir.AluOpType
    X = mybir.AxisListType.X

    nc = bacc.Bacc(num_devices=NCORES, debug=False)

    # host-prepped inputs (bf16 unless noted)
    # xp is pre-arranged on the host into device tile layout
    # [128, NT*(DIN+1)] so its DMA is one contiguous run per partition.
    xp_d = nc.dram_tensor("xp", [128, NT * (DIN + 1)], bf16, kind="ExternalInput")
    xt_d = nc.dram_tensor("xT", [DIN, R], bf16, kind="ExternalInput")
    w1t_d = nc.dram_tensor("w1t", [DIN, H], bf16, kind="ExternalInput")
    w2t_d = nc.dram_tensor("w2t", [H, H], bf16, kind="ExternalInput")
    wl1t_d = nc.dram_tensor("wl1t", [H, HM], bf16, kind="ExternalInput")
    wl2t_d = nc.dram_tensor("wl2t", [HM, C], bf16, kind="ExternalInput")
    # f32 param columns: 0-7 g1, 8-15 be1, 16-23 g2, 24-31 be2, 32-33 bl1,
    # col 34 rows 0-9 = bl2
    pc_d = nc.dram_tensor("pcols", [128, 35], fp32, kind="ExternalInput")
    out_d = nc.dram_tensor("out", [R, C], fp32, kind="ExternalOutput")

    RG = [list(range(NCORES))]

    with tile.TileContext(nc) as tc:
        with (
            tc.tile_pool(name="persist", bufs=1) as persist,
            tc.tile_pool(name="work", bufs=2) as work,
            tc.tile_pool(name="dram", bufs=1, space="DRAM") as dram,
        ):
            # ---------------- persistent tiles -----------------
            identity = persist.tile([128, 128], fp32, tag="identity", name="identity")
            make_identity(nc, identity[:])
            ones_col = persist.tile([128, 1], bf16, tag="ones", name="ones_col")
            nc.vector.memset(ones_col[:], 1.0)

            pcols = persist.tile([128, 35], fp32, tag="pcols", name="pcols")
            xT = persist.tile([128, R], bf16, tag="xT", name="xT")
            w1T = persist.tile([128, H], bf16, tag="w1T", name="w1T")
            w2T = [
                persist.tile([128, H], bf16, tag=f"w2T{k}", name=f"w2T{k}")
                for k in range(8)
            ]
            wl1T = [
                persist.tile([128, HM], bf16, tag=f"wl1T{k}", name=f"wl1T{k}")
                for k in range(8)
            ]
            wl2T = [
                persist.tile([128, C], bf16, tag=f"wl2T{k}", name=f"wl2T{k}")
                for k in range(2)
            ]
            bn1_scale = persist.tile([128, 8], fp32, tag="bn1s", name="bn1_scale")
            bn1_bias = persist.tile([128, 8], fp32, tag="bn1b", name="bn1_bias")
            h2_sb = [
                persist.tile([128, R], bf16, tag=f"h2sb{m}", name=f"h2sb{m}")
                for m in range(8)
            ]
            # BN2 stat partials: col s*NCH+j, s=m for sums, s=8+m for sumsq
            stats_parts = persist.tile(
                [128, 16 * NCH], fp32, tag="statp", name="stats_parts"
            )

            # collective scratch (DRAM). BN1 stats are projected locally
            # (wxm/e2 are linear in the per-core Gram) so both collectives
            # carry only [128,16] instead of the raw [128,129] Gram.
            cc1_in = dram.tile([128, 16], fp32, name="cc1_in")
            cc1_out = dram.tile([NCORES, 128, 16], fp32, name="cc1_out")
            cc2_in = dram.tile([128, 16], fp32, name="cc2_in")
            cc2_out = dram.tile([NCORES, 128, 16], fp32, name="cc2_out")

            # ============ startup: loads + Gram + AllReduce#1 ============
            with tc.tile_pool(name="bigload", bufs=1) as bigload, \
                 tc.tile_pool(name="pacc", bufs=1, space="PSUM") as pacc, \
                 tc.tile_pool(name="pl1", bufs=3, space="PSUM") as pl1, \
                 tc.tile_pool(name="pst", bufs=2, space="PSUM") as pst, \
                 tc.tile_pool(name="pv", bufs=2, space="PSUM") as pv:
                # pre-warm the ACT sqrt table (sqrt_and_others also holds
                # Relu/Identity) so the BN1-stats sqrt doesn't pay the
                # 1.3us ACT_TABLE_LOAD on the critical path
                warm = work.tile([128, 1], fp32, tag="warm", name="warm")
                nc.scalar.activation(warm[:], identity[:, 0:1], AF.Sqrt)

                # x (with ones col) in pieces on sync/scalar queues; gram
                # chases it. gpsimd queue is reserved for collectives.
                D1 = DIN + 1
                xall = bigload.tile([128, NT * D1], bf16, tag="xall", name="xall")
                npieces = 6
                x_engs = [nc.sync, nc.scalar]
                step = (NT + npieces - 1) // npieces
                for bi in range(npieces):
                    ta, tb = bi * step, min((bi + 1) * step, NT)
                    if ta >= tb:
                        continue
                    x_engs[bi % 2].dma_start(
                        out=xall[:, ta * D1 : tb * D1],
                        in_=xp_d[:, ta * D1 : tb * D1],
                    )

                # ---- other loads (sync/scalar queues only) ----
                nc.scalar.dma_start(out=w1T[:], in_=w1t_d[:])
                quarter = (R // 6 + 511) // 512 * 512
                for qi in range(6):
                    qa, qb = qi * quarter, min((qi + 1) * quarter, R)
                    if qa < qb:
                        x_engs[qi % 2].dma_start(out=xT[:, qa:qb], in_=xt_d[:, qa:qb])
                nc.scalar.dma_start(out=pcols[:], in_=pc_d[:])
                for k in range(2):
                    nc.sync.dma_start(
                        out=wl2T[k][:], in_=wl2t_d[k * 128 : (k + 1) * 128, :]
                    )

                gram_ps = pacc.tile([128, D1], fp32, tag="gram", name="gram_ps")
                for i in range(NT):
                    nc.tensor.matmul(
                        gram_ps[:],
                        lhsT=xall[:, i * D1 : i * D1 + DIN],
                        rhs=xall[:, i * D1 : (i + 1) * D1],
                        start=(i == 0),
                        stop=(i == NT - 1),
                    )
                # ---- project local Gram to BN1 stat contributions ----
                # (wxm = W1 @ colsum; e2 = colsum_din((G @ W1T) * W1T);
                # both are linear in G, so project first, reduce after)
                gram_bf = work.tile([128, D1], bf16, tag="grambf", name="gram_bf")
                nc.scalar.copy(gram_bf[:], gram_ps[:])
                wxm_ps = pst.tile([128, 8], fp32, tag="pst", name="wxm_ps")
                for m in range(8):
                    nc.tensor.matmul(
                        wxm_ps[:, m : m + 1],
                        lhsT=w1T[:, m * 128 : (m + 1) * 128],
                        rhs=gram_bf[:, DIN : DIN + 1],
                        start=True,
                        stop=True,
                    )
                V_sb = work.tile([128, H], bf16, tag="Vsb", name="V_sb")
                for hh in range(2):
                    vp = pv.tile([128, 512], fp32, tag="pv", name=f"vp{hh}")
                    nc.tensor.matmul(
                        vp[:],
                        lhsT=gram_bf[:, 0:DIN],
                        rhs=w1T[:, hh * 512 : (hh + 1) * 512],
                        start=True,
                        stop=True,
                    )
                    nc.vector.tensor_mul(
                        V_sb[:, hh * 512 : (hh + 1) * 512],
                        w1T[:, hh * 512 : (hh + 1) * 512],
                        vp[:],
                    )
                e2_ps = pst.tile([128, 8], fp32, tag="pst", name="e2_ps")
                for m in range(8):
                    nc.tensor.matmul(
                        e2_ps[:, m : m + 1],
                        lhsT=V_sb[:, m * 128 : (m + 1) * 128],
                        rhs=ones_col[:],
                        start=True,
                        stop=True,
                    )
                stats1_sb = work.tile([128, 16], fp32, tag="st1", name="stats1_sb")
                nc.vector.tensor_copy(stats1_sb[:, 0:8], wxm_ps[:])
                nc.vector.tensor_copy(stats1_sb[:, 8:16], e2_ps[:])
                # input copy on sync queue, collective on gpsimd
                nc.sync.dma_start(out=cc1_in[:], in_=stats1_sb[:])
                nc.gpsimd.collective_compute(
                    "AllGather",
                    ALU.bypass,
                    replica_groups=RG,
                    ins=[cc1_in[:].opt()],
                    outs=[cc1_out[:].opt()],
                )

                # big weight loads issued from the gpsimd engine AFTER the
                # collective: the engine blocks on the collective instruction,
                # so these descriptors hit the DMA queues only once the ring
                # transfers are done — they'd otherwise delay AllReduce#1
                # (shared HW DMA queues drain in order). w2T is needed at L2,
                # wl1T at the final pass; both arrive in time.
                for k in range(8):
                    nc.gpsimd.dma_start(
                        out=w2T[k][:], in_=w2t_d[k * 128 : (k + 1) * 128, :]
                    )
                for k in range(8):
                    nc.gpsimd.dma_start(
                        out=wl1T[k][:], in_=wl1t_d[k * 128 : (k + 1) * 128, :]
                    )

                # ---- run-ahead L1: raw h1 for all chunks, staged as bf16
                # into h2_sb's storage (overwritten by real h2 later). Keeps
                # the PE busy while AllReduce#1 is in flight; BN1-apply later
                # reads the staged h1.
                for j, (c0, cc) in enumerate(CH_LIST):
                    for m in range(8):
                        h1_ps = pl1.tile(
                            [128, CH], fp32, tag="pl1", name=f"h1st{j}_{m}"
                        )
                        nc.tensor.matmul(
                            h1_ps[:, :cc],
                            lhsT=w1T[:, m * 128 : (m + 1) * 128],
                            rhs=xT[:, c0 : c0 + cc],
                            start=True,
                            stop=True,
                        )
                        eng = nc.scalar if (j * 8 + m) % 2 else nc.vector
                        if eng is nc.scalar:
                            nc.scalar.copy(
                                h2_sb[m][:, c0 : c0 + cc], h1_ps[:, :cc]
                            )
                        else:
                            nc.vector.tensor_copy(
                                h2_sb[m][:, c0 : c0 + cc], h1_ps[:, :cc]
                            )

                # ---- BN1 statistics: gather + tree-reduce + affine ----
                ag1 = work.tile([128, NCORES * 16], fp32, tag="ag1", name="ag1")
                nc.sync.dma_start(
                    out=ag1[:].rearrange("p (g d) -> p g d", d=16),
                    in_=cc1_out[:].rearrange("g p d -> p g d"),
                )
                nc.vector.tensor_add(ag1[:, :64], ag1[:, :64], ag1[:, 64:])
                nc.vector.tensor_add(ag1[:, :32], ag1[:, :32], ag1[:, 32:64])
                stats1g = work.tile([128, 16], fp32, tag="st1g", name="stats1g")
                nc.vector.tensor_add(stats1g[:], ag1[:, :16], ag1[:, 16:32])
                meanh = work.tile([128, 8], fp32, tag="meanh", name="meanh")
                nc.vector.tensor_scalar_mul(meanh[:], stats1g[:, 0:8], 1.0 / N_TOTAL)
                e2n = work.tile([128, 8], fp32, tag="e2n", name="e2n")
                nc.scalar.mul(e2n[:], stats1g[:, 8:16], 1.0 / N_TOTAL)
                msq = work.tile([128, 8], fp32, tag="msq", name="msq")
                nc.vector.tensor_mul(msq[:], meanh[:], meanh[:])
                var1 = work.tile([128, 8], fp32, tag="var1", name="var1")
                nc.vector.tensor_sub(var1[:], e2n[:], msq[:])
                nc.vector.tensor_scalar_add(var1[:], var1[:], BN_EPS)
                sd1 = work.tile([128, 8], fp32, tag="sd1", name="sd1")
                nc.scalar.sqrt(sd1[:], var1[:])
                rstd = work.tile([128, 8], fp32, tag="rstd", name="rstd")
                nc.vector.reciprocal(rstd[:], sd1[:])
                nc.vector.tensor_mul(bn1_scale[:], rstd[:], pcols[:, 0:8])
                t2 = work.tile([128, 8], fp32, tag="t2", name="t2")
                nc.vector.tensor_mul(t2[:], meanh[:], bn1_scale[:])
                nc.vector.tensor_sub(bn1_bias[:], pcols[:, 8:16], t2[:])

            if stage == "s1":
                dummy = work.tile([128, C], fp32, tag="dummy", name="dummy")
                nc.vector.tensor_copy(dummy[:, 0:8], bn1_scale[:])
                nc.vector.tensor_copy(dummy[:, 8:10], bn1_bias[:, 0:2])
                for r0 in range(0, R, 128):
                    rr = min(128, R - r0)
                    nc.sync.dma_start(out=out_d[r0 : r0 + rr, :], in_=dummy[:rr, :])
            else:
                _build_rest(
                    nc, tc, stage, mybir, fp32, bf16, AF, ALU, X,
                    persist, work, dram, identity, pcols,
                    xT, w1T, w2T, wl1T, wl2T, h2_sb, stats_parts,
                    bn1_scale, bn1_bias, cc2_in, cc2_out, out_d, RG,
                )

    nc.finalize()
    # NOTE: deduplicating redundant same-weights InstLdweights was tried
    # and measured SLOWER (454us vs 426us): the ~50ns inter-matmul gap is
    # per-instruction overhead, not a weight-swap drain, and the removed
    # ld slots had been overlapping the next load. Keep V_DEDUP off.
    if os.environ.get("V_DEDUP", "0") == "1":
        _dedup_ldweights(nc.m)
    return nc


def _dedup_ldweights(m):
    """Drop redundant InstLdweights that reload the exact weights already
    in the PE array. Measured: do not use (see note at call site)."""
    n_drop = 0
    for fn in m.functions:
        for blk in fn.blocks:
            insts = blk.instructions
            drop = []
            prev_sig = None
            for idx in range(len(insts)):
                inst = insts[idx]
                if "PE" not in str(inst.engine):
                    continue
                tn = type(inst).__name__
                if tn == "InstLdweights":
                    try:
                        sig = str(inst.ins[0])
                    except Exception:
                        sig = None
                    si = inst.sync_info
                    clean = si is None or (
                        len(si.on_wait) == 0 and len(si.on_update) == 0
                    )
                    if sig is not None and sig == prev_sig and clean:
                        drop.append(idx)
                    else:
                        prev_sig = sig
                elif tn == "InstMatmult":
                    pass  # matmul leaves loaded weights untouched
                else:
                    prev_sig = None
            for idx in reversed(drop):
                del insts[idx]
            n_drop += len(drop)


def _build_rest(
    nc, tc, stage, mybir, fp32, bf16, AF, ALU, X,
    persist, work, dram, identity, pcols,
    xT, w1T, w2T, wl1T, wl2T, h2_sb, stats_parts,
    bn1_scale, bn1_bias, cc2_in, cc2_out, out_d, RG,
):
    # ------------- main pass: L1 -> BN1+ReLU -> L2 (h2 to SBUF) ------------
    # The tail chunk goes FIRST: its 106-col cleanup ops then never gate
    # the AR2 trigger — the last-finishing chunk is a full-width one whose
    # stats ops complete at pipeline pace right behind its last matmul.
    with (
        tc.tile_pool(name="acts", bufs=1) as acts,
        tc.tile_pool(name="sqs", bufs=3) as sqs,
        tc.tile_pool(name="ph2", bufs=4, space="PSUM") as ph2,
    ):
        main_groups = [GROUPS[-1]] + GROUPS[:-1]
        for gi_m, grp in enumerate(main_groups):
            a1 = {}
            for pi, (j, (c0, cc)) in enumerate(
                [(CH_LIST.index(ch), ch) for ch in grp]
            ):
                a1[j] = [
                    acts.tile(
                        [128, CH],
                        bf16,
                        tag=f"act{gi_m & 1}_{pi}_{k}",
                        name=f"a1_{j}_{k}",
                    )
                    for k in range(8)
                ]
            pair = [(CH_LIST.index(ch), ch) for ch in grp]
            # BN1-apply from staged h1 (in h2_sb storage)
            for m in range(8):
                for j, (c0, cc) in pair:
                    nc.scalar.activation(
                        a1[j][m][:, :cc],
                        h2_sb[m][:, c0 : c0 + cc],
                        AF.Relu,
                        bias=bn1_bias[:, m : m + 1],
                        scale=bn1_scale[:, m : m + 1],
                    )
            # L2
            for m in range(8):
                sl = slice(m * 128, (m + 1) * 128)
                h2_ps = {}
                for j, (c0, cc) in pair:
                    h2_ps[j] = ph2.tile(
                        [128, CH], fp32, tag="ph2", name=f"h2ps{j}_{m}"
                    )
                for k in range(8):
                    for j, (c0, cc) in pair:
                        nc.tensor.matmul(
                            h2_ps[j][:, :cc],
                            lhsT=w2T[k][:, sl],
                            rhs=a1[j][k][:, :cc],
                            start=(k == 0),
                            stop=(k == 7),
                        )
                for j, (c0, cc) in pair:
                    sum_slot = stats_parts[:, m * NCH + j : m * NCH + j + 1]
                    sq_slot = stats_parts[
                        :, (8 + m) * NCH + j : (8 + m) * NCH + j + 1
                    ]
                    # h2 copy + per-feature row-sum, ACT/DVE alternating
                    # (tensor_tensor_reduce fails walrus codegen; use
                    # ACT-accum or explicit DVE copy + reduce instead).
                    sq = sqs.tile([128, CH], bf16, tag="sq", name=f"sq{j}_{m}")
                    if j == NCH - 1:
                        # tail chunk feeds the AR2 trigger: split its 106-col
                        # ops evenly across ACT and DVE so neither queue
                        # serializes the collective start
                        if m % 2 == 0:
                            nc.scalar.activation(
                                h2_sb[m][:, c0 : c0 + cc],
                                h2_ps[j][:, :cc],
                                AF.Identity,
                                bias=0.0,
                                scale=1.0,
                                accum_out=sum_slot,
                            )
                            nc.vector.tensor_mul(
                                sq[:, :cc],
                                h2_sb[m][:, c0 : c0 + cc],
                                h2_sb[m][:, c0 : c0 + cc],
                            )
                            nc.vector.reduce_sum(sq_slot, sq[:, :cc], axis=X)
                        else:
                            nc.vector.tensor_copy(
                                h2_sb[m][:, c0 : c0 + cc], h2_ps[j][:, :cc]
                            )
                            nc.vector.reduce_sum(
                                sum_slot, h2_sb[m][:, c0 : c0 + cc], axis=X
                            )
                            nc.scalar.activation(
                                sq[:, :cc],
                                h2_sb[m][:, c0 : c0 + cc],
                                AF.Square,
                                bias=0.0,
                                scale=1.0,
                                accum_out=sq_slot,
                            )
                        continue
                    if (m + j) % 2 == 0:
                        nc.scalar.activation(
                            h2_sb[m][:, c0 : c0 + cc],
                            h2_ps[j][:, :cc],
                            AF.Identity,
                            bias=0.0,
                            scale=1.0,
                            accum_out=sum_slot,
                        )
                    else:
                        nc.vector.tensor_copy(
                            h2_sb[m][:, c0 : c0 + cc], h2_ps[j][:, :cc]
                        )
                        nc.vector.reduce_sum(
                            sum_slot, h2_sb[m][:, c0 : c0 + cc], axis=X
                        )
                    # sumsq: Pool squares + DVE reduce, or DVE mul + reduce.
                    # Chunks 10/11 are processed last (tail chunk runs
                    # first): keep their squares off the slow Pool queue so
                    # its backlog never gates the AR2 trigger.
                    if (m + j) % 2 == 0 and j < NCH - 3:
                        nc.gpsimd.tensor_tensor(
                            sq[:, :cc],
                            h2_sb[m][:, c0 : c0 + cc],
                            h2_sb[m][:, c0 : c0 + cc],
                            ALU.mult,
                        )
                    else:
                        nc.vector.tensor_mul(
                            sq[:, :cc],
                            h2_sb[m][:, c0 : c0 + cc],
                            h2_sb[m][:, c0 : c0 + cc],
                        )
                    nc.vector.reduce_sum(sq_slot, sq[:, :cc], axis=X)

    # ---------------- BN2 statistics ----------------
    stats2_sb = work.tile([128, 16], fp32, tag="st2", name="stats2_sb")
    nc.vector.reduce_sum(
        stats2_sb[:],
        stats_parts[:].rearrange("p (s j) -> p s j", j=NCH),
        axis=X,
    )
    nc.sync.dma_start(out=cc2_in[:], in_=stats2_sb[:])
    stats2g = work.tile([128, 16], fp32, tag="st2g", name="stats2g")
    nc.gpsimd.collective_compute(
        "AllGather",
        ALU.bypass,
        replica_groups=RG,
        ins=[cc2_in[:].opt()],
        outs=[cc2_out[:].opt()],
    )
    # copy on the scalar engine's HW DMA queue (gpsimd's software queue
    # adds ~2.4us completion latency on this critical path)
    ag2 = work.tile([128, NCORES * 16], fp32, tag="ag2", name="ag2")
    nc.scalar.dma_start(
        out=ag2[:].rearrange("p (g d) -> p g d", d=16),
        in_=cc2_out[:].rearrange("g p d -> p g d"),
    )
    nc.vector.tensor_add(ag2[:, :64], ag2[:, :64], ag2[:, 64:])
    nc.vector.tensor_add(ag2[:, :32], ag2[:, :32], ag2[:, 32:64])
    nc.vector.tensor_add(stats2g[:], ag2[:, :16], ag2[:, 16:32])

    bn2_scale = persist.tile([128, 8], fp32, tag="bn2s", name="bn2_scale")
    bn2_bias = persist.tile([128, 8], fp32, tag="bn2b", name="bn2_bias")
    mean2 = work.tile([128, 8], fp32, tag="mean2", name="mean2")
    nc.vector.tensor_scalar_mul(mean2[:], stats2g[:, 0:8], 1.0 / N_TOTAL)
    e2n2 = work.tile([128, 8], fp32, tag="e2n2", name="e2n2")
    nc.scalar.mul(e2n2[:], stats2g[:, 8:16], 1.0 / N_TOTAL)
    msq2 = work.tile([128, 8], fp32, tag="msq2", name="msq2")
    nc.vector.tensor_mul(msq2[:], mean2[:], mean2[:])
    var2 = work.tile([128, 8], fp32, tag="var2", name="var2")
    nc.vector.tensor_sub(var2[:], e2n2[:], msq2[:])
    nc.vector.tensor_scalar_add(var2[:], var2[:], BN_EPS)
    sd2 = work.tile([128, 8], fp32, tag="sd2", name="sd2")
    nc.scalar.sqrt(sd2[:], var2[:])
    rstd2 = work.tile([128, 8], fp32, tag="rstd2", name="rstd2")
    nc.vector.reciprocal(rstd2[:], sd2[:])
    nc.vector.tensor_mul(bn2_scale[:], rstd2[:], pcols[:, 16:24])
    t22 = work.tile([128, 8], fp32, tag="t22", name="t22")
    nc.vector.tensor_mul(t22[:], mean2[:], bn2_scale[:])
    nc.vector.tensor_sub(bn2_bias[:], pcols[:, 24:32], t22[:])

    if stage == "s2":
        dummy = work.tile([128, C], fp32, tag="dummy", name="dummy")
        nc.vector.tensor_copy(dummy[:, 0:8], bn2_scale[:])
        nc.vector.tensor_copy(dummy[:, 8:10], bn2_bias[:, 0:2])
        for r0 in range(0, R, 128):
            rr = min(128, R - r0)
            nc.sync.dma_start(out=out_d[r0 : r0 + rr, :], in_=dummy[:rr, :])
        return

    # ------ final pass: BN2+ReLU -> L3 -> L4 -> softmax -> store ------
    with (
        tc.tile_pool(name="acts2", bufs=1) as acts2,
        tc.tile_pool(name="sqs2", bufs=2) as sqs2,
        tc.tile_pool(name="h3pool", bufs=1) as h3pool,
        tc.tile_pool(name="lgpool", bufs=1) as lgpool,
        tc.tile_pool(name="smpool", bufs=2) as smpool,
        tc.tile_pool(name="ph3", bufs=3, space="PSUM") as ph3,
        tc.tile_pool(name="plog", bufs=1, space="PSUM") as plog,
        tc.tile_pool(name="ptr2", bufs=2, space="PSUM") as ptr2,
    ):
        for gi, grp in enumerate(GROUPS):
            pair = [(CH_LIST.index(ch), ch) for ch in grp]
            p0 = pair[0][1][0]
            W = sum(cc for _, (c0, cc) in pair)  # 1024 or 106
            # BN2-apply (ACT-heavy; DVE and GPSIMD elementwise are slower).
            # First group applies in 512-wide halves so the first L3
            # matmul starts ~0.7us sooner after the collective.
            a2 = [
                acts2.tile(
                    [128, 2 * CH], bf16, tag=f"a2_{gi & 1}_{k}", name=f"a2_{gi}_{k}"
                )
                for k in range(8)
            ]
            halves = (
                [(0, CH), (CH, W - CH)] if (gi == 0 and W > CH) else [(0, W)]
            )
            for k in range(8):
                if k < 5:
                    for ha, hw in halves:
                        nc.scalar.activation(
                            a2[k][:, ha : ha + hw],
                            h2_sb[k][:, p0 + ha : p0 + ha + hw],
                            AF.Relu,
                            bias=bn2_bias[:, k : k + 1],
                            scale=bn2_scale[:, k : k + 1],
                        )
                else:
                    tmp = sqs2.tile(
                        [128, 2 * CH], bf16, tag="tmp2", name=f"t2_{gi}_{k}"
                    )
                    for ha, hw in halves:
                        nc.vector.tensor_scalar(
                            out=tmp[:, ha : ha + hw],
                            in0=h2_sb[k][:, p0 + ha : p0 + ha + hw],
                            scalar1=bn2_scale[:, k : k + 1],
                            scalar2=bn2_bias[:, k : k + 1],
                            op0=ALU.mult,
                            op1=ALU.add,
                        )
                        nc.vector.tensor_scalar_max(
                            a2[k][:, ha : ha + hw], tmp[:, ha : ha + hw], 0.0
                        )
            # L3 per chunk
            h3 = {}
            for j, (c0, cc) in pair:
                off = c0 - p0
                h3[j] = [
                    h3pool.tile(
                        [128, CH], bf16, tag=f"h3_{j & 1}_{m3}", name=f"h3_{j}_{m3}"
                    )
                    for m3 in range(2)
                ]
                for m3 in range(2):
                    sl = slice(m3 * 128, (m3 + 1) * 128)
                    h3_ps = ph3.tile([128, CH], fp32, tag="ph3", name=f"h3ps{j}_{m3}")
                    for k in range(8):
                        nc.tensor.matmul(
                            h3_ps[:, :cc],
                            lhsT=wl1T[k][:, sl],
                            rhs=a2[k][:, off : off + cc],
                            start=(k == 0),
                            stop=(k == 7),
                        )
                    nc.vector.tensor_scalar(
                        out=h3[j][m3][:, :cc],
                        in0=h3_ps[:, :cc],
                        scalar1=pcols[:, 32 + m3 : 33 + m3],
                        scalar2=0.0,
                        op0=ALU.add,
                        op1=ALU.max,
                    )
            # L4 per chunk -> feature-major logits for the group
            lg_sb = lgpool.tile([C, 2 * CH], fp32, tag="lg", name=f"lg{gi}")
            for j, (c0, cc) in pair:
                off = c0 - p0
                lg_ps = plog.tile([C, CH], fp32, tag=f"plog{j & 1}", name=f"lgps{j}")
                for k2 in range(2):
                    nc.tensor.matmul(
                        lg_ps[:, :cc],
                        lhsT=wl2T[k2][:],
                        rhs=h3[j][k2][:, :cc],
                        start=(k2 == 0),
                        stop=(k2 == 1),
                    )
                nc.vector.tensor_scalar_add(
                    lg_sb[:, off : off + cc], lg_ps[:, :cc], pcols[0:C, 34:35]
                )
            # transpose logits to row-major, softmax, store
            ntile = (W + 127) // 128
            rows_sb = smpool.tile([128, 8 * C], fp32, tag="rows", name=f"rows{gi}")
            for t in range(ntile):
                rt = min(128, W - t * 128)
                tp_ps = ptr2.tile([128, C], fp32, tag="ptr2", name=f"tp{gi}_{t}")
                nc.tensor.transpose(
                    tp_ps[:rt, :],
                    lg_sb[:, t * 128 : t * 128 + rt],
                    identity[:C, :C],
                )
                nc.vector.tensor_copy(rows_sb[:rt, t * C : (t + 1) * C], tp_ps[:rt, :])
            e_sb = smpool.tile([128, 8 * C], fp32, tag="esb", name=f"e{gi}")
            sums = smpool.tile([128, 8], fp32, tag="sums", name=f"sums{gi}")
            lse = smpool.tile([128, 8], fp32, tag="lse", name=f"lse{gi}")
            res = smpool.tile([128, 8 * C], fp32, tag="res", name=f"res{gi}")
            nw = ntile * C
            nc.scalar.activation(e_sb[:, :nw], rows_sb[:, :nw], AF.Exp)
            nc.vector.reduce_sum(
                sums[:, :ntile],
                e_sb[:, :nw].rearrange("p (t c) -> p t c", c=C),
                axis=X,
            )
            # lse = ln(sums) via DVE bit-trick (exponent extract + cubic on
            # the mantissa, max err ~9e-4). Keeps AF.Ln off the ACT engine:
            # Exp/Ln live in different ACT tables, so per-group Ln forced 2
            # table reloads (2.6us) per group, ~2.6us of it pure tail.
            i32 = mybir.dt.int32
            iv = sums[:, :ntile].bitcast(i32)
            u32 = smpool.tile([128, 8], i32, tag="u32", name=f"u32_{gi}")
            nc.vector.tensor_scalar(
                out=u32[:, :ntile], in0=iv, scalar1=23, scalar2=0x4B000000,
                op0=ALU.arith_shift_right, op1=ALU.bitwise_or,
            )
            ef = smpool.tile([128, 8], fp32, tag="ef", name=f"ef_{gi}")
            nc.vector.tensor_scalar_sub(
                ef[:, :ntile], u32[:, :ntile].bitcast(fp32), 8388735.0
            )
            mi = smpool.tile([128, 8], i32, tag="mi", name=f"mi_{gi}")
            nc.vector.tensor_scalar(
                out=mi[:, :ntile], in0=iv, scalar1=0x007FFFFF,
                scalar2=0x3F800000,
                op0=ALU.bitwise_and, op1=ALU.bitwise_or,
            )
            mf = mi[:, :ntile].bitcast(fp32)
            pp = smpool.tile([128, 8], fp32, tag="pp", name=f"pp_{gi}")
            nc.vector.tensor_scalar(
                out=pp[:, :ntile], in0=mf,
                scalar1=0.10668473, scalar2=-0.71359,
                op0=ALU.mult, op1=ALU.add,
            )
            nc.vector.tensor_mul(pp[:, :ntile], pp[:, :ntile], mf)
            nc.vector.tensor_scalar_add(pp[:, :ntile], pp[:, :ntile], 2.08687922)
            nc.vector.tensor_mul(pp[:, :ntile], pp[:, :ntile], mf)
            nc.vector.tensor_scalar_add(pp[:, :ntile], pp[:, :ntile], -1.47904892)
            nc.vector.tensor_scalar_mul(
                ef[:, :ntile], ef[:, :ntile], 0.6931471805599453
            )
            nc.vector.tensor_add(lse[:, :ntile], pp[:, :ntile], ef[:, :ntile])
            nc.vector.tensor_sub(
                res[:, :nw].rearrange("p (t c) -> p t c", c=C),
                rows_sb[:, :nw].rearrange("p (t c) -> p t c", c=C),
                lse[:, :ntile].to_broadcast([128, ntile, C]),
            )
            # store: full tiles contiguous, split across issue queues
            # (sync HW queue + gpsimd; scalar-queue stores interleave DMA
            # issue ops with the ACT compute stream in the final pass)
            nfull = W // 128
            st_engs = [nc.sync, nc.gpsimd, nc.sync, nc.gpsimd]
            for si in range(min(4, nfull)):
                ta = si * nfull // 4 if nfull >= 4 else si
                tb = (si + 1) * nfull // 4 if nfull >= 4 else si + 1
                if ta >= tb:
                    continue
                st_engs[si].dma_start(
                    out=out_d[p0 + ta * 128 : p0 + tb * 128].rearrange(
                        "(t p) c -> p t c", p=128
                    ),
                    in_=res[:, ta * C : tb * C].rearrange("p (t c) -> p t c", c=C),
                )
            rtail = W - nfull * 128
            if rtail:
                nc.sync.dma_start(
                    out=out_d[p0 + nfull * 128 : p0 + W],
                    in_=res[:rtail, nfull * C : nfull * C + C],
                )


def _get_nc():
    if "nc" not in _CACHE:
        _CACHE["nc"] = _build(os.environ.get("KERNEL_STAGE", "full"))
    return _CACHE["nc"]


def prep_in_maps(inputs):
    import ml_dtypes

    f32 = np.float32
    bf = ml_dtypes.bfloat16
    x = np.ascontiguousarray(np.asarray(inputs["x"]), dtype=f32)
    W1 = np.asarray(inputs["W1"], dtype=f32)
    W2 = np.asarray(inputs["W2"], dtype=f32)
    Wl1 = np.asarray(inputs["Wl1"], dtype=f32)
    Wl2 = np.asarray(inputs["Wl2"], dtype=f32)

    w1t = np.ascontiguousarray(W1.T).astype(bf)
    w2t = np.ascontiguousarray(W2.T).astype(bf)
    wl1t = np.ascontiguousarray(Wl1.T).astype(bf)
    wl2t = np.ascontiguousarray(Wl2.T).astype(bf)

    pcols = np.zeros((128, 35), f32)
    for grp, nm in enumerate(["g1", "be1", "g2", "be2"]):
        v = np.asarray(inputs[nm], dtype=f32)
        for m in range(8):
            pcols[:, grp * 8 + m] = v[m * 128 : (m + 1) * 128]
    bl1 = np.asarray(inputs["bl1"], dtype=f32)
    pcols[:, 32] = bl1[0:128]
    pcols[:, 33] = bl1[128:256]
    pcols[0:C, 34] = np.asarray(inputs["bl2"], dtype=f32)

    in_maps = []
    for i in range(NCORES):
        xs = x[i * R : (i + 1) * R]
        xp = np.zeros((RPAD, DIN + 1), bf)
        xp[:R, :DIN] = xs.astype(bf)
        xp[:R, DIN] = bf(1.0)
        # device tile layout: [128, NT*(DIN+1)], partition-major
        xp_dev = np.ascontiguousarray(
            xp.reshape(NT, 128, DIN + 1).transpose(1, 0, 2).reshape(128, -1)
        )
        in_maps.append(
            {
                "xp": xp_dev,
                "xT": np.ascontiguousarray(xs.T).astype(bf),
                "w1t": w1t,
                "w2t": w2t,
                "wl1t": wl1t,
                "wl2t": wl2t,
                "pcols": pcols,
            }
        )
    return in_maps


def kernel(**inputs):
    from concourse.bass_utils import run_bass_kernel_spmd

    nc = _get_nc()
    in_maps = prep_in_maps(inputs)
    res = run_bass_kernel_spmd(nc, in_maps, core_ids=list(range(NCORES)))
    return np.concatenate([r["out"] for r in res.results], axis=0).astype(np.float32)


# revision 38
# speedup vs baseline: 1.0531x; 1.0531x over previous
"""Trainium2 Bass kernel for ChebyNet (K=1) forward pass.

ChebConv with K=1 reduces to a plain linear layer on the T0 (identity) term,
so edge_index / edge_weight never enter the math. The network is:

    h1 = x @ W1.T (+b1; cancels in BN) -> BN (train, over nodes) -> ReLU
    h2 = h1 @ W2.T (+b2; cancels)      -> BN -> ReLU
    h3 = relu(h2 @ Wl1.T + bl1)
    out = log_softmax(h3 @ Wl2.T + bl2, axis=1)

Sharding: nodes (N=50000) split across 8 NeuronCores (6250 rows each).
All compute is node-local except BN statistics:
  - BN1 stats come analytically from the Gram matrix of [x | 1] (one
    AllReduce of [128,129] f32 + local math).
  - BN2 stats from per-shard sum/sumsq of h2: AllGather [128,16] + reduce.

v6 changes vs v2 (434925 ns baseline):
  - BN1 stats projected locally before the collective (wxm/e2 are
    linear in the per-core Gram), shrinking collective #1 from the raw
    [128,129] Gram (66KB AllReduce RDH, 14.9us) to [128,16] (8KB
    AllGather Mesh, ~6.4us) + local tree-reduce.
  - ph2 bufs 2->4: cross-m PSUM double-buffering puts the L2 phase at
    its 263ns/matmul cadence floor (p90 delta 264ns).
  - main-phase engine rebalance: h2 copy+sum split ACT/DVE, squares
    split Pool(gpsimd)/DVE; the tail chunk that feeds the AR2 trigger
    splits ACT/DVE so no single queue serializes the collective start.
  - BN2 stat partials laid out in one tile -> single final reduce op;
    AllGather + tree-reduce instead of Mesh AllReduce (~4us); the
    gathered-output copy rides the scalar engine's HW DMA queue (the
    gpsimd software queue adds ~2.4us completion latency).
  - ACT sqrt table pre-warmed at startup (saves a 1.3us table load on
    the BN1 critics path); first group's BN2-apply runs in 512-wide
    halves so L3 starts sooner after the collective.
  - output stores ride the sync+scalar HW DMA queues (the gpsimd
    software queue added a multi-us DRAIN to the epilogue).

Known dead ends (measured): fp8 fails the 2e-2 gate (2.7e-2 L2-only);
a warm-up dummy collective serializes in front of AR1 instead of
hiding the rendezvous barrier (+16us); GPSIMD elementwise BN-apply is
2.6 cyc/elem + steals the DVE SBUF port (final pass 3x slower);
deferring ALL stores past the last group exposes the ~4GB/s/queue
store drain (+24us); matmul outputs cannot span PSUM banks (512 f32
cols max); tensor_tensor_reduce fails walrus codegen.
"""

import os
import sys

sys.path.insert(0, "/opt/trn_rl_repo")

import numpy as np

NCORES = 8
N_TOTAL = 50000
R = N_TOTAL // NCORES  # 6250 rows per core
DIN = 128
H = 1024
HM = 256
C = 10
BN_EPS = 1e-5
CH = 512
NT = (R + 127) // 128  # 49 row tiles (for Gram); host pads x to NT*128 rows
RPAD = NT * 128  # 6272

CH_LIST = [(i * CH, min(CH, R - i * CH)) for i in range((R + CH - 1) // CH)]
NCH = len(CH_LIST)  # 13
# groups of two chunks (last group is the lone tail chunk)
GROUPS = [CH_LIST[i : i + 2] for i in range(0, NCH, 2)]

_CACHE = {}


def _build(stage="full"):
    import concourse.bass as bass  # noqa: F401
    import concourse.tile as tile
    import concourse.mybir as mybir
    from concourse import bacc
    from concourse.masks import make_identity

    fp32 = mybir.dt.float32
    bf16 = mybir.dt.bfloat16
    AF = mybir.ActivationFunctionType
    ALU = mybir.AluOpType
    X = mybir.AxisListType.X

    nc = bacc.Bacc(num_devices=NCORES, debug=False)

    # host-prepped inputs (bf16 unless noted)
    # xp is pre-arranged on the host into device tile layout
    # [128, NT*(DIN+1)] so its DMA is one contiguous run per partition.
    xp_d = nc.dram_tensor("xp", [128, NT * (DIN + 1)], bf16, kind="ExternalInput")
    xt_d = nc.dram_tensor("xT", [DIN, R], bf16, kind="ExternalInput")
    w1t_d = nc.dram_tensor("w1t", [DIN, H], bf16, kind="ExternalInput")
    w2t_d = nc.dram_tensor("w2t", [H, H], bf16, kind="ExternalInput")
    wl1t_d = nc.dram_tensor("wl1t", [H, HM], bf16, kind="ExternalInput")
    wl2t_d = nc.dram_tensor("wl2t", [HM, C], bf16, kind="ExternalInput")
    # f32 param columns: 0-7 g1, 8-15 be1, 16-23 g2, 24-31 be2, 32-33 bl1,
    # col 34 rows 0-9 = bl2
    pc_d = nc.dram_tensor("pcols", [128, 35], fp32, kind="ExternalInput")
    out_d = nc.dram_tensor("out", [R, C], fp32, kind="ExternalOutput")

    RG = [list(range(NCORES))]

    with tile.TileContext(nc) as tc:
        with (
            tc.tile_pool(name="persist", bufs=1) as persist,
            tc.tile_pool(name="work", bufs=2) as work,
            tc.tile_pool(name="dram", bufs=1, space="DRAM") as dram,
        ):
            # ---------------- persistent tiles -----------------
            identity = persist.tile([128, 128], fp32, tag="identity", name="identity")
            make_identity(nc, identity[:])
            ones_col = persist.tile([128, 1], bf16, tag="ones", name="ones_col")
            nc.vector.memset(ones_col[:], 1.0)

            pcols = persist.tile([128, 35], fp32, tag="pcols", name="pcols")
            xT = persist.tile([128, R], bf16, tag="xT", name="xT")
            w1T = persist.tile([128, H], bf16, tag="w1T", name="w1T")
            w2T = [
                persist.tile([128, H], bf16, tag=f"w2T{k}", name=f"w2T{k}")
                for k in range(8)
            ]
            wl1T = [
                persist.tile([128, HM], bf16, tag=f"wl1T{k}", name=f"wl1T{k}")
                for k in range(8)
            ]
            wl2T = [
                persist.tile([128, C], bf16, tag=f"wl2T{k}", name=f"wl2T{k}")
                for k in range(2)
            ]
            bn1_scale = persist.tile([128, 8], fp32, tag="bn1s", name="bn1_scale")
            bn1_bias = persist.tile([128, 8], fp32, tag="bn1b", name="bn1_bias")
            h2_sb = [
                persist.tile([128, R], bf16, tag=f"h2sb{m}", name=f"h2sb{m}")
                for m in range(8)
            ]
            # BN2 stat partials: col s*NCH+j, s=m for sums, s=8+m for sumsq
            stats_parts = persist.tile(
                [128, 16 * NCH], fp32, tag="statp", name="stats_parts"
            )

            # collective scratch (DRAM). BN1 stats are projected locally
            # (wxm/e2 are linear in the per-core Gram) so both collectives
            # carry only [128,16] instead of the raw [128,129] Gram.
            cc1_in = dram.tile([128, 16], fp32, name="cc1_in")
            cc1_out = dram.tile([NCORES, 128, 16], fp32, name="cc1_out")
            cc2_in = dram.tile([128, 16], fp32, name="cc2_in")
            cc2_out = dram.tile([NCORES, 128, 16], fp32, name="cc2_out")

            # ============ startup: loads + Gram + AllReduce#1 ============
            with tc.tile_pool(name="bigload", bufs=1) as bigload, \
                 tc.tile_pool(name="pacc", bufs=1, space="PSUM") as pacc, \
                 tc.tile_pool(name="pl1", bufs=3, space="PSUM") as pl1, \
                 tc.tile_pool(name="pst", bufs=2, space="PSUM") as pst, \
                 tc.tile_pool(name="pv", bufs=2, space="PSUM") as pv:
                # pre-warm the ACT sqrt table (sqrt_and_others also holds
                # Relu/Identity) so the BN1-stats sqrt doesn't pay the
                # 1.3us ACT_TABLE_LOAD on the critical path
                warm = work.tile([128, 1], fp32, tag="warm", name="warm")
                nc.scalar.activation(warm[:], identity[:, 0:1], AF.Sqrt)

                # x (with ones col) in pieces on sync/scalar queues; gram
                # chases it. gpsimd queue is reserved for collectives.
                D1 = DIN + 1
                xall = bigload.tile([128, NT * D1], bf16, tag="xall", name="xall")
                npieces = 6
                x_engs = [nc.sync, nc.scalar]
                step = (NT + npieces - 1) // npieces
                for bi in range(npieces):
                    ta, tb = bi * step, min((bi + 1) * step, NT)
                    if ta >= tb:
                        continue
                    x_engs[bi % 2].dma_start(
                        out=xall[:, ta * D1 : tb * D1],
                        in_=xp_d[:, ta * D1 : tb * D1],
                    )

                # ---- other loads (sync/scalar queues only) ----
                nc.scalar.dma_start(out=w1T[:], in_=w1t_d[:])
                quarter = (R // 6 + 511) // 512 * 512
                for qi in range(6):
                    qa, qb = qi * quarter, min((qi + 1) * quarter, R)
                    if qa < qb:
                        x_engs[qi % 2].dma_start(out=xT[:, qa:qb], in_=xt_d[:, qa:qb])
                nc.scalar.dma_start(out=pcols[:], in_=pc_d[:])
                for k in range(2):
                    nc.sync.dma_start(
                        out=wl2T[k][:], in_=wl2t_d[k * 128 : (k + 1) * 128, :]
                    )

                gram_ps = pacc.tile([128, D1], fp32, tag="gram", name="gram_ps")
                for i in range(NT):
                    nc.tensor.matmul(
                        gram_ps[:],
                        lhsT=xall[:, i * D1 : i * D1 + DIN],
                        rhs=xall[:, i * D1 : (i + 1) * D1],
                        start=(i == 0),
                        stop=(i == NT - 1),
                    )
                # ---- project local Gram to BN1 stat contributions ----
                # (wxm = W1 @ colsum; e2 = colsum_din((G @ W1T) * W1T);
                # both are linear in G, so project first, reduce after)
                gram_bf = work.tile([128, D1], bf16, tag="grambf", name="gram_bf")
                nc.scalar.copy(gram_bf[:], gram_ps[:])
                wxm_ps = pst.tile([128, 8], fp32, tag="pst", name="wxm_ps")
                for m in range(8):
                    nc.tensor.matmul(
                        wxm_ps[:, m : m + 1],
                        lhsT=w1T[:, m * 128 : (m + 1) * 128],
                        rhs=gram_bf[:, DIN : DIN + 1],
                        start=True,
                        stop=True,
                    )
                V_sb = work.tile([128, H], bf16, tag="Vsb", name="V_sb")
                for hh in range(2):
                    vp = pv.tile([128, 512], fp32, tag="pv", name=f"vp{hh}")
                    nc.tensor.matmul(
                        vp[:],
                        lhsT=gram_bf[:, 0:DIN],
                        rhs=w1T[:, hh * 512 : (hh + 1) * 512],
                        start=True,
                        stop=True,
                    )
                    nc.vector.tensor_mul(
                        V_sb[:, hh * 512 : (hh + 1) * 512],
                        w1T[:, hh * 512 : (hh + 1) * 512],
                        vp[:],
                    )
                e2_ps = pst.tile([128, 8], fp32, tag="pst", name="e2_ps")
                for m in range(8):
                    nc.tensor.matmul(
                        e2_ps[:, m : m + 1],
                        lhsT=V_sb[:, m * 128 : (m + 1) * 128],
                        rhs=ones_col[:],
                        start=True,
                        stop=True,
                    )
                stats1_sb = work.tile([128, 16], fp32, tag="st1", name="stats1_sb")
                nc.vector.tensor_copy(stats1_sb[:, 0:8], wxm_ps[:])
                nc.vector.tensor_copy(stats1_sb[:, 8:16], e2_ps[:])
                # input copy on sync queue, collective on gpsimd
                nc.sync.dma_start(out=cc1_in[:], in_=stats1_sb[:])
                nc.gpsimd.collective_compute(
                    "AllGather",
                    ALU.bypass,
                    replica_groups=RG,
                    ins=[cc1_in[:].opt()],
                    outs=[cc1_out[:].opt()],
                )

                # big weight loads issued from the gpsimd engine AFTER the
                # collective: the engine blocks on the collective instruction,
                # so these descriptors hit the DMA queues only once the ring
                # transfers are done — they'd otherwise delay AllReduce#1
                # (shared HW DMA queues drain in order). w2T is needed at L2,
                # wl1T at the final pass; both arrive in time.
                for k in range(8):
                    nc.gpsimd.dma_start(
                        out=w2T[k][:], in_=w2t_d[k * 128 : (k + 1) * 128, :]
                    )
                for k in range(8):
                    nc.gpsimd.dma_start(
                        out=wl1T[k][:], in_=wl1t_d[k * 128 : (k + 1) * 128, :]
                    )

                # ---- run-ahead L1: raw h1 for all chunks, staged as bf16
                # into h2_sb's storage (overwritten by real h2 later). Keeps
                # the PE busy while AllReduce#1 is in flight; BN1-apply later
                # reads the staged h1.
                for j, (c0, cc) in enumerate(CH_LIST):
                    for m in range(8):
                        h1_ps = pl1.tile(
                            [128, CH], fp32, tag="pl1", name=f"h1st{j}_{m}"
                        )
                        nc.tensor.matmul(
                            h1_ps[:, :cc],
                            lhsT=w1T[:, m * 128 : (m + 1) * 128],
                            rhs=xT[:, c0 : c0 + cc],
                            start=True,
                            stop=True,
                        )
                        eng = nc.scalar if (j * 8 + m) % 2 else nc.vector
                        if eng is nc.scalar:
                            nc.scalar.copy(
                                h2_sb[m][:, c0 : c0 + cc], h1_ps[:, :cc]
                            )
                        else:
                            nc.vector.tensor_copy(
                                h2_sb[m][:, c0 : c0 + cc], h1_ps[:, :cc]
                            )

                # ---- BN1 statistics: gather + tree-reduce + affine ----
                ag1 = work.tile([128, NCORES * 16], fp32, tag="ag1", name="ag1")
                nc.sync.dma_start(
                    out=ag1[:].rearrange("p (g d) -> p g d", d=16),
                    in_=cc1_out[:].rearrange("g p d -> p g d"),
                )
                # PE p-state warm-up: a few junk matmuls gated on the
                # gathered stats run during the tree/affine window, so the
                # first real L2 matmuls start at max clock instead of
                # ramping from a ~37us idle. Sized (12 x ~265ns) well under
                # the ~4.4us post-collective math latency.
                wsrc = work.tile([128, 128], bf16, tag="wsrc", name="wsrc")
                nc.vector.memset(wsrc[:], 0.0)
                nc.vector.tensor_copy(wsrc[:, 0:1], ag1[:, 0:1])
                for wi in range(10):
                    wps = pl1.tile([128, CH], fp32, tag="pl1", name=f"warm1_{wi}")
                    nc.tensor.matmul(
                        wps[:, :128],
                        lhsT=w1T[:, 0:128],
                        rhs=wsrc[:],
                        start=True,
                        stop=True,
                    )
                nc.vector.tensor_add(ag1[:, :64], ag1[:, :64], ag1[:, 64:])
                nc.vector.tensor_add(ag1[:, :32], ag1[:, :32], ag1[:, 32:64])
                stats1g = work.tile([128, 16], fp32, tag="st1g", name="stats1g")
                nc.vector.tensor_add(stats1g[:], ag1[:, :16], ag1[:, 16:32])
                meanh = work.tile([128, 8], fp32, tag="meanh", name="meanh")
                nc.vector.tensor_scalar_mul(meanh[:], stats1g[:, 0:8], 1.0 / N_TOTAL)
                e2n = work.tile([128, 8], fp32, tag="e2n", name="e2n")
                nc.scalar.mul(e2n[:], stats1g[:, 8:16], 1.0 / N_TOTAL)
                msq = work.tile([128, 8], fp32, tag="msq", name="msq")
                nc.vector.tensor_mul(msq[:], meanh[:], meanh[:])
                var1 = work.tile([128, 8], fp32, tag="var1", name="var1")
                nc.vector.tensor_sub(var1[:], e2n[:], msq[:])
                nc.vector.tensor_scalar_add(var1[:], var1[:], BN_EPS)
                sd1 = work.tile([128, 8], fp32, tag="sd1", name="sd1")
                nc.scalar.sqrt(sd1[:], var1[:])
                rstd = work.tile([128, 8], fp32, tag="rstd", name="rstd")
                nc.vector.reciprocal(rstd[:], sd1[:])
                nc.vector.tensor_mul(bn1_scale[:], rstd[:], pcols[:, 0:8])
                t2 = work.tile([128, 8], fp32, tag="t2", name="t2")
                nc.vector.tensor_mul(t2[:], meanh[:], bn1_scale[:])
                nc.vector.tensor_sub(bn1_bias[:], pcols[:, 8:16], t2[:])

            if stage == "s1":
                dummy = work.tile([128, C], fp32, tag="dummy", name="dummy")
                nc.vector.tensor_copy(dummy[:, 0:8], bn1_scale[:])
                nc.vector.tensor_copy(dummy[:, 8:10], bn1_bias[:, 0:2])
                for r0 in range(0, R, 128):
                    rr = min(128, R - r0)
                    nc.sync.dma_start(out=out_d[r0 : r0 + rr, :], in_=dummy[:rr, :])
            else:
                _build_rest(
                    nc, tc, stage, mybir, fp32, bf16, AF, ALU, X,
                    persist, work, dram, identity, pcols,
                    xT, w1T, w2T, wl1T, wl2T, h2_sb, stats_parts,
                    bn1_scale, bn1_bias, cc2_in, cc2_out, out_d, RG,
                )

    nc.finalize()
    # NOTE: deduplicating redundant same-weights InstLdweights was tried
    # and measured SLOWER (454us vs 426us): the ~50ns inter-matmul gap is
    # per-instruction overhead, not a weight-swap drain, and the removed
    # ld slots had been overlapping the next load. Keep V_DEDUP off.
    if os.environ.get("V_DEDUP", "0") == "1":
        _dedup_ldweights(nc.m)
    return nc


def _dedup_ldweights(m):
    """Drop redundant InstLdweights that reload the exact weights already
    in the PE array. Measured: do not use (see note at call site)."""
    n_drop = 0
    for fn in m.functions:
        for blk in fn.blocks:
            insts = blk.instructions
            drop = []
            prev_sig = None
            for idx in range(len(insts)):
                inst = insts[idx]
                if "PE" not in str(inst.engine):
                    continue
                tn = type(inst).__name__
                if tn == "InstLdweights":
                    try:
                        sig = str(inst.ins[0])
                    except Exception:
                        sig = None
                    si = inst.sync_info
                    clean = si is None or (
                        len(si.on_wait) == 0 and len(si.on_update) == 0
                    )
                    if sig is not None and sig == prev_sig and clean:
                        drop.append(idx)
                    else:
                        prev_sig = sig
                elif tn == "InstMatmult":
                    pass  # matmul leaves loaded weights untouched
                else:
                    prev_sig = None
            for idx in reversed(drop):
                del insts[idx]
            n_drop += len(drop)


def _build_rest(
    nc, tc, stage, mybir, fp32, bf16, AF, ALU, X,
    persist, work, dram, identity, pcols,
    xT, w1T, w2T, wl1T, wl2T, h2_sb, stats_parts,
    bn1_scale, bn1_bias, cc2_in, cc2_out, out_d, RG,
):
    # ------------- main pass: L1 -> BN1+ReLU -> L2 (h2 to SBUF) ------------
    # The tail chunk goes FIRST: its 106-col cleanup ops then never gate
    # the AR2 trigger — the last-finishing chunk is a full-width one whose
    # stats ops complete at pipeline pace right behind its last matmul.
    with (
        tc.tile_pool(name="acts", bufs=1) as acts,
        tc.tile_pool(name="sqs", bufs=3) as sqs,
        tc.tile_pool(name="ph2", bufs=4, space="PSUM") as ph2,
    ):
        main_groups = [GROUPS[-1]] + GROUPS[:-1]
        for gi_m, grp in enumerate(main_groups):
            a1 = {}
            for pi, (j, (c0, cc)) in enumerate(
                [(CH_LIST.index(ch), ch) for ch in grp]
            ):
                a1[j] = [
                    acts.tile(
                        [128, CH],
                        bf16,
                        tag=f"act{gi_m & 1}_{pi}_{k}",
                        name=f"a1_{j}_{k}",
                    )
                    for k in range(8)
                ]
            pair = [(CH_LIST.index(ch), ch) for ch in grp]
            # BN1-apply from staged h1 (in h2_sb storage)
            for m in range(8):
                for j, (c0, cc) in pair:
                    nc.scalar.activation(
                        a1[j][m][:, :cc],
                        h2_sb[m][:, c0 : c0 + cc],
                        AF.Relu,
                        bias=bn1_bias[:, m : m + 1],
                        scale=bn1_scale[:, m : m + 1],
                    )
            # L2
            for m in range(8):
                sl = slice(m * 128, (m + 1) * 128)
                h2_ps = {}
                for j, (c0, cc) in pair:
                    h2_ps[j] = ph2.tile(
                        [128, CH], fp32, tag="ph2", name=f"h2ps{j}_{m}"
                    )
                for k in range(8):
                    for j, (c0, cc) in pair:
                        nc.tensor.matmul(
                            h2_ps[j][:, :cc],
                            lhsT=w2T[k][:, sl],
                            rhs=a1[j][k][:, :cc],
                            start=(k == 0),
                            stop=(k == 7),
                        )
                for j, (c0, cc) in pair:
                    sum_slot = stats_parts[:, m * NCH + j : m * NCH + j + 1]
                    sq_slot = stats_parts[
                        :, (8 + m) * NCH + j : (8 + m) * NCH + j + 1
                    ]
                    # h2 copy + per-feature row-sum, ACT/DVE alternating
                    # (tensor_tensor_reduce fails walrus codegen; use
                    # ACT-accum or explicit DVE copy + reduce instead).
                    sq = sqs.tile([128, CH], bf16, tag="sq", name=f"sq{j}_{m}")
                    if j == NCH - 1:
                        # tail chunk feeds the AR2 trigger: split its 106-col
                        # ops evenly across ACT and DVE so neither queue
                        # serializes the collective start
                        if m % 2 == 0:
                            nc.scalar.activation(
                                h2_sb[m][:, c0 : c0 + cc],
                                h2_ps[j][:, :cc],
                                AF.Identity,
                                bias=0.0,
                                scale=1.0,
                                accum_out=sum_slot,
                            )
                            nc.vector.tensor_mul(
                                sq[:, :cc],
                                h2_sb[m][:, c0 : c0 + cc],
                                h2_sb[m][:, c0 : c0 + cc],
                            )
                            nc.vector.reduce_sum(sq_slot, sq[:, :cc], axis=X)
                        else:
                            nc.vector.tensor_copy(
                                h2_sb[m][:, c0 : c0 + cc], h2_ps[j][:, :cc]
                            )
                            nc.vector.reduce_sum(
                                sum_slot, h2_sb[m][:, c0 : c0 + cc], axis=X
                            )
                            nc.scalar.activation(
                                sq[:, :cc],
                                h2_sb[m][:, c0 : c0 + cc],
                                AF.Square,
                                bias=0.0,
                                scale=1.0,
                                accum_out=sq_slot,
                            )
                        continue
                    if (m + j) % 2 == 0:
                        nc.scalar.activation(
                            h2_sb[m][:, c0 : c0 + cc],
                            h2_ps[j][:, :cc],
                            AF.Identity,
                            bias=0.0,
                            scale=1.0,
                            accum_out=sum_slot,
                        )
                    else:
                        nc.vector.tensor_copy(
                            h2_sb[m][:, c0 : c0 + cc], h2_ps[j][:, :cc]
                        )
                        nc.vector.reduce_sum(
                            sum_slot, h2_sb[m][:, c0 : c0 + cc], axis=X
                        )
                    # sumsq: Pool squares + DVE reduce, or DVE mul + reduce.
                    # Chunks 10/11 are processed last (tail chunk runs
                    # first): keep their squares off the slow Pool queue AND
                    # split them ACT/DVE (ACT has no next-group BN1-apply at
                    # phase end) so no queue backlog gates the AR2 trigger.
                    if j >= NCH - 3 and j != NCH - 1:
                        if (m + j) % 2 == 0:
                            nc.scalar.activation(
                                sq[:, :cc],
                                h2_sb[m][:, c0 : c0 + cc],
                                AF.Square,
                                bias=0.0,
                                scale=1.0,
                                accum_out=sq_slot,
                            )
                        else:
                            nc.vector.tensor_mul(
                                sq[:, :cc],
                                h2_sb[m][:, c0 : c0 + cc],
                                h2_sb[m][:, c0 : c0 + cc],
                            )
                            nc.vector.reduce_sum(sq_slot, sq[:, :cc], axis=X)
                        continue
                    if (m + j) % 2 == 0:
                        nc.gpsimd.tensor_tensor(
                            sq[:, :cc],
                            h2_sb[m][:, c0 : c0 + cc],
                            h2_sb[m][:, c0 : c0 + cc],
                            ALU.mult,
                        )
                    else:
                        nc.vector.tensor_mul(
                            sq[:, :cc],
                            h2_sb[m][:, c0 : c0 + cc],
                            h2_sb[m][:, c0 : c0 + cc],
                        )
                    nc.vector.reduce_sum(sq_slot, sq[:, :cc], axis=X)

    # ---------------- BN2 statistics ----------------
    stats2_sb = work.tile([128, 16], fp32, tag="st2", name="stats2_sb")
    nc.vector.reduce_sum(
        stats2_sb[:],
        stats_parts[:].rearrange("p (s j) -> p s j", j=NCH),
        axis=X,
    )
    nc.sync.dma_start(out=cc2_in[:], in_=stats2_sb[:])
    stats2g = work.tile([128, 16], fp32, tag="st2g", name="stats2g")
    nc.gpsimd.collective_compute(
        "AllGather",
        ALU.bypass,
        replica_groups=RG,
        ins=[cc2_in[:].opt()],
        outs=[cc2_out[:].opt()],
    )
    # copy on the scalar engine's HW DMA queue (gpsimd's software queue
    # adds ~2.4us completion latency on this critical path)
    ag2 = work.tile([128, NCORES * 16], fp32, tag="ag2", name="ag2")
    nc.scalar.dma_start(
        out=ag2[:].rearrange("p (g d) -> p g d", d=16),
        in_=cc2_out[:].rearrange("g p d -> p g d"),
    )
    # PE p-state warm-up during the BN2 math window (see AR1 warm-up):
    # 16 x ~265ns of junk matmuls gated on the gathered stats, under the
    # ~7us post-collective latency so the first L3 matmul isn't delayed.
    with tc.tile_pool(name="pwarm", bufs=2, space="PSUM") as pwarm:
        wsrc2 = work.tile([128, 128], bf16, tag="wsrc2", name="wsrc2")
        nc.vector.memset(wsrc2[:], 0.0)
        nc.vector.tensor_copy(wsrc2[:, 0:1], ag2[:, 0:1])
        for wi in range(16):
            wps2 = pwarm.tile([128, CH], fp32, tag="pw", name=f"warm2_{wi}")
            nc.tensor.matmul(
                wps2[:, :128],
                lhsT=w2T[0][:, 0:128],
                rhs=wsrc2[:],
                start=True,
                stop=True,
            )
    nc.vector.tensor_add(ag2[:, :64], ag2[:, :64], ag2[:, 64:])
    nc.vector.tensor_add(ag2[:, :32], ag2[:, :32], ag2[:, 32:64])
    nc.vector.tensor_add(stats2g[:], ag2[:, :16], ag2[:, 16:32])

    bn2_scale = persist.tile([128, 8], fp32, tag="bn2s", name="bn2_scale")
    bn2_bias = persist.tile([128, 8], fp32, tag="bn2b", name="bn2_bias")
    mean2 = work.tile([128, 8], fp32, tag="mean2", name="mean2")
    nc.vector.tensor_scalar_mul(mean2[:], stats2g[:, 0:8], 1.0 / N_TOTAL)
    e2n2 = work.tile([128, 8], fp32, tag="e2n2", name="e2n2")
    nc.scalar.mul(e2n2[:], stats2g[:, 8:16], 1.0 / N_TOTAL)
    msq2 = work.tile([128, 8], fp32, tag="msq2", name="msq2")
    nc.vector.tensor_mul(msq2[:], mean2[:], mean2[:])
    var2 = work.tile([128, 8], fp32, tag="var2", name="var2")
    nc.vector.tensor_sub(var2[:], e2n2[:], msq2[:])
    nc.vector.tensor_scalar_add(var2[:], var2[:], BN_EPS)
    sd2 = work.tile([128, 8], fp32, tag="sd2", name="sd2")
    nc.scalar.sqrt(sd2[:], var2[:])
    rstd2 = work.tile([128, 8], fp32, tag="rstd2", name="rstd2")
    nc.vector.reciprocal(rstd2[:], sd2[:])
    nc.vector.tensor_mul(bn2_scale[:], rstd2[:], pcols[:, 16:24])
    t22 = work.tile([128, 8], fp32, tag="t22", name="t22")
    nc.vector.tensor_mul(t22[:], mean2[:], bn2_scale[:])
    nc.vector.tensor_sub(bn2_bias[:], pcols[:, 24:32], t22[:])

    if stage == "s2":
        dummy = work.tile([128, C], fp32, tag="dummy", name="dummy")
        nc.vector.tensor_copy(dummy[:, 0:8], bn2_scale[:])
        nc.vector.tensor_copy(dummy[:, 8:10], bn2_bias[:, 0:2])
        for r0 in range(0, R, 128):
            rr = min(128, R - r0)
            nc.sync.dma_start(out=out_d[r0 : r0 + rr, :], in_=dummy[:rr, :])
        return

    # ------ final pass: BN2+ReLU -> L3 -> L4 -> softmax -> store ------
    with (
        tc.tile_pool(name="acts2", bufs=1) as acts2,
        tc.tile_pool(name="sqs2", bufs=2) as sqs2,
        tc.tile_pool(name="h3pool", bufs=1) as h3pool,
        tc.tile_pool(name="lgpool", bufs=1) as lgpool,
        tc.tile_pool(name="smpool", bufs=2) as smpool,
        tc.tile_pool(name="ph3", bufs=3, space="PSUM") as ph3,
        tc.tile_pool(name="plog", bufs=1, space="PSUM") as plog,
        tc.tile_pool(name="ptr2", bufs=2, space="PSUM") as ptr2,
    ):
        for gi, grp in enumerate(GROUPS):
            pair = [(CH_LIST.index(ch), ch) for ch in grp]
            p0 = pair[0][1][0]
            W = sum(cc for _, (c0, cc) in pair)  # 1024 or 106
            # BN2-apply (ACT-heavy; DVE and GPSIMD elementwise are slower).
            # First group applies in 512-wide halves so the first L3
            # matmul starts ~0.7us sooner after the collective.
            a2 = [
                acts2.tile(
                    [128, 2 * CH], bf16, tag=f"a2_{gi & 1}_{k}", name=f"a2_{gi}_{k}"
                )
                for k in range(8)
            ]
            halves = (
                [(0, CH), (CH, W - CH)] if (gi == 0 and W > CH) else [(0, W)]
            )
            for k in range(8):
                if k < 5:
                    for ha, hw in halves:
                        nc.scalar.activation(
                            a2[k][:, ha : ha + hw],
                            h2_sb[k][:, p0 + ha : p0 + ha + hw],
                            AF.Relu,
                            bias=bn2_bias[:, k : k + 1],
                            scale=bn2_scale[:, k : k + 1],
                        )
                else:
                    tmp = sqs2.tile(
                        [128, 2 * CH], bf16, tag="tmp2", name=f"t2_{gi}_{k}"
                    )
                    for ha, hw in halves:
                        nc.vector.tensor_scalar(
                            out=tmp[:, ha : ha + hw],
                            in0=h2_sb[k][:, p0 + ha : p0 + ha + hw],
                            scalar1=bn2_scale[:, k : k + 1],
                            scalar2=bn2_bias[:, k : k + 1],
                            op0=ALU.mult,
                            op1=ALU.add,
                        )
                        nc.vector.tensor_scalar_max(
                            a2[k][:, ha : ha + hw], tmp[:, ha : ha + hw], 0.0
                        )
            # L3 per chunk
            h3 = {}
            for j, (c0, cc) in pair:
                off = c0 - p0
                h3[j] = [
                    h3pool.tile(
                        [128, CH], bf16, tag=f"h3_{j & 1}_{m3}", name=f"h3_{j}_{m3}"
                    )
                    for m3 in range(2)
                ]
                for m3 in range(2):
                    sl = slice(m3 * 128, (m3 + 1) * 128)
                    h3_ps = ph3.tile([128, CH], fp32, tag="ph3", name=f"h3ps{j}_{m3}")
                    for k in range(8):
                        nc.tensor.matmul(
                            h3_ps[:, :cc],
                            lhsT=wl1T[k][:, sl],
                            rhs=a2[k][:, off : off + cc],
                            start=(k == 0),
                            stop=(k == 7),
                        )
                    nc.vector.tensor_scalar(
                        out=h3[j][m3][:, :cc],
                        in0=h3_ps[:, :cc],
                        scalar1=pcols[:, 32 + m3 : 33 + m3],
                        scalar2=0.0,
                        op0=ALU.add,
                        op1=ALU.max,
                    )
            # L4 per chunk -> feature-major logits for the group
            lg_sb = lgpool.tile([C, 2 * CH], fp32, tag="lg", name=f"lg{gi}")
            for j, (c0, cc) in pair:
                off = c0 - p0
                lg_ps = plog.tile([C, CH], fp32, tag=f"plog{j & 1}", name=f"lgps{j}")
                for k2 in range(2):
                    nc.tensor.matmul(
                        lg_ps[:, :cc],
                        lhsT=wl2T[k2][:],
                        rhs=h3[j][k2][:, :cc],
                        start=(k2 == 0),
                        stop=(k2 == 1),
                    )
                nc.vector.tensor_scalar_add(
                    lg_sb[:, off : off + cc], lg_ps[:, :cc], pcols[0:C, 34:35]
                )
            # transpose logits to row-major, softmax, store
            ntile = (W + 127) // 128
            rows_sb = smpool.tile([128, 8 * C], fp32, tag="rows", name=f"rows{gi}")
            for t in range(ntile):
                rt = min(128, W - t * 128)
                tp_ps = ptr2.tile([128, C], fp32, tag="ptr2", name=f"tp{gi}_{t}")
                nc.tensor.transpose(
                    tp_ps[:rt, :],
                    lg_sb[:, t * 128 : t * 128 + rt],
                    identity[:C, :C],
                )
                nc.vector.tensor_copy(rows_sb[:rt, t * C : (t + 1) * C], tp_ps[:rt, :])
            e_sb = smpool.tile([128, 8 * C], fp32, tag="esb", name=f"e{gi}")
            sums = smpool.tile([128, 8], fp32, tag="sums", name=f"sums{gi}")
            lse = smpool.tile([128, 8], fp32, tag="lse", name=f"lse{gi}")
            res = smpool.tile([128, 8 * C], fp32, tag="res", name=f"res{gi}")
            nw = ntile * C
            nc.scalar.activation(e_sb[:, :nw], rows_sb[:, :nw], AF.Exp)
            nc.vector.reduce_sum(
                sums[:, :ntile],
                e_sb[:, :nw].rearrange("p (t c) -> p t c", c=C),
                axis=X,
            )
            # lse = ln(sums) via DVE bit-trick (exponent extract + cubic on
            # the mantissa, max err ~9e-4). Keeps AF.Ln off the ACT engine:
            # Exp/Ln live in different ACT tables, so per-group Ln forced 2
            # table reloads (2.6us) per group, ~2.6us of it pure tail.
            i32 = mybir.dt.int32
            iv = sums[:, :ntile].bitcast(i32)
            u32 = smpool.tile([128, 8], i32, tag="u32", name=f"u32_{gi}")
            nc.vector.tensor_scalar(
                out=u32[:, :ntile], in0=iv, scalar1=23, scalar2=0x4B000000,
                op0=ALU.arith_shift_right, op1=ALU.bitwise_or,
            )
            ef = smpool.tile([128, 8], fp32, tag="ef", name=f"ef_{gi}")
            nc.vector.tensor_scalar_sub(
                ef[:, :ntile], u32[:, :ntile].bitcast(fp32), 8388735.0
            )
            mi = smpool.tile([128, 8], i32, tag="mi", name=f"mi_{gi}")
            nc.vector.tensor_scalar(
                out=mi[:, :ntile], in0=iv, scalar1=0x007FFFFF,
                scalar2=0x3F800000,
                op0=ALU.bitwise_and, op1=ALU.bitwise_or,
            )
            mf = mi[:, :ntile].bitcast(fp32)
            pp = smpool.tile([128, 8], fp32, tag="pp", name=f"pp_{gi}")
            nc.vector.tensor_scalar(
                out=pp[:, :ntile], in0=mf,
                scalar1=0.10668473, scalar2=-0.71359,
                op0=ALU.mult, op1=ALU.add,
            )
            nc.vector.tensor_mul(pp[:, :ntile], pp[:, :ntile], mf)
            nc.vector.tensor_scalar_add(pp[:, :ntile], pp[:, :ntile], 2.08687922)
            nc.vector.tensor_mul(pp[:, :ntile], pp[:, :ntile], mf)
            nc.vector.tensor_scalar_add(pp[:, :ntile], pp[:, :ntile], -1.47904892)
            nc.vector.tensor_scalar_mul(
                ef[:, :ntile], ef[:, :ntile], 0.6931471805599453
            )
            nc.vector.tensor_add(lse[:, :ntile], pp[:, :ntile], ef[:, :ntile])
            nc.vector.tensor_sub(
                res[:, :nw].rearrange("p (t c) -> p t c", c=C),
                rows_sb[:, :nw].rearrange("p (t c) -> p t c", c=C),
                lse[:, :ntile].to_broadcast([128, ntile, C]),
            )
            # store: full tiles contiguous, split across issue queues
            # (sync HW queue + gpsimd; scalar-queue stores interleave DMA
            # issue ops with the ACT compute stream in the final pass)
            nfull = W // 128
            st_engs = [nc.sync, nc.gpsimd, nc.sync, nc.gpsimd]
            for si in range(min(4, nfull)):
                ta = si * nfull // 4 if nfull >= 4 else si
                tb = (si + 1) * nfull // 4 if nfull >= 4 else si + 1
                if ta >= tb:
                    continue
                st_engs[si].dma_start(
                    out=out_d[p0 + ta * 128 : p0 + tb * 128].rearrange(
                        "(t p) c -> p t c", p=128
                    ),
                    in_=res[:, ta * C : tb * C].rearrange("p (t c) -> p t c", c=C),
                )
            rtail = W - nfull * 128
            if rtail:
                nc.sync.dma_start(
                    out=out_d[p0 + nfull * 128 : p0 + W],
                    in_=res[:rtail, nfull * C : nfull * C + C],
                )


def _get_nc():
    if "nc" not in _CACHE:
        _CACHE["nc"] = _build(os.environ.get("KERNEL_STAGE", "full"))
    return _CACHE["nc"]


def prep_in_maps(inputs):
    import ml_dtypes

    f32 = np.float32
    bf = ml_dtypes.bfloat16
    x = np.ascontiguousarray(np.asarray(inputs["x"]), dtype=f32)
    W1 = np.asarray(inputs["W1"], dtype=f32)
    W2 = np.asarray(inputs["W2"], dtype=f32)
    Wl1 = np.asarray(inputs["Wl1"], dtype=f32)
    Wl2 = np.asarray(inputs["Wl2"], dtype=f32)

    w1t = np.ascontiguousarray(W1.T).astype(bf)
    w2t = np.ascontiguousarray(W2.T).astype(bf)
    wl1t = np.ascontiguousarray(Wl1.T).astype(bf)
    wl2t = np.ascontiguousarray(Wl2.T).astype(bf)

    pcols = np.zeros((128, 35), f32)
    for grp, nm in enumerate(["g1", "be1", "g2", "be2"]):
        v = np.asarray(inputs[nm], dtype=f32)
        for m in range(8):
            pcols[:, grp * 8 + m] = v[m * 128 : (m + 1) * 128]
    bl1 = np.asarray(inputs["bl1"], dtype=f32)
    pcols[:, 32] = bl1[0:128]
    pcols[:, 33] = bl1[128:256]
    pcols[0:C, 34] = np.asarray(inputs["bl2"], dtype=f32)

    in_maps = []
    for i in range(NCORES):
        xs = x[i * R : (i + 1) * R]
        xp = np.zeros((RPAD, DIN + 1), bf)
        xp[:R, :DIN] = xs.astype(bf)
        xp[:R, DIN] = bf(1.0)
        # device tile layout: [128, NT*(DIN+1)], partition-major
        xp_dev = np.ascontiguousarray(
            xp.reshape(NT, 128, DIN + 1).transpose(1, 0, 2).reshape(128, -1)
        )
        in_maps.append(
            {
                "xp": xp_dev,
                "xT": np.ascontiguousarray(xs.T).astype(bf),
                "w1t": w1t,
                "w2t": w2t,
                "wl1t": wl1t,
                "wl2t": wl2t,
                "pcols": pcols,
            }
        )
    return in_maps


def kernel(**inputs):
    from concourse.bass_utils import run_bass_kernel_spmd

    nc = _get_nc()
    in_maps = prep_in_maps(inputs)
    res = run_bass_kernel_spmd(nc, in_maps, core_ids=list(range(NCORES)))
    return np.concatenate([r["out"] for r in res.results], axis=0).astype(np.float32)
